# revision 13
# baseline (speedup 1.0000x reference)
"""Trainium2 Bass kernel for nn_Decoder (GRU decoder + Bahdanau attention + vocab proj).

Contract: kernel(**full_inputs) -> (logits (T,B,V) f32, hT (1,B,H) f32)
Internally: 8-core SPMD. Data-parallel over B for attention/output (2 batches
per core); the GRU scan is replicated full-B on every core (its PE cost is
batch-independent, so replication buys zero-communication); the vocab
projection is row-sharded (each core: its 2 batches x full V, streaming
ws_w.T in bf16).

Per-core batch permutation trick: core c receives its inputs with batch order
[2c, 2c+1, rest...] so the device program always works on local batches 0,1 —
one SPMD program, no rank-dependent addressing.
"""

import os
import sys

sys.path.insert(0, "/opt/trn_rl_repo")
os.environ.setdefault("MYCRO_LOCAL_CACHE", "1")

import numpy as np
import ml_dtypes

import concourse.bass as bass
import concourse.mybir as mybir
import concourse.tile as tile
from concourse import bacc
from concourse.bass import IndirectOffsetOnAxis, ts
from concourse.bass_utils import run_bass_kernel_spmd
from concourse.masks import make_identity

F32 = mybir.dt.float32
F32R = mybir.dt.float32r
BF16 = mybir.dt.bfloat16
I32 = mybir.dt.int32
AF = mybir.ActivationFunctionType
ALU = mybir.AluOpType
AX = mybir.AxisListType

T, B, S, H, E, V = 64, 16, 128, 512, 512, 32000
NCORES = 8
BL = 2              # batches per core
RB = T * B          # 1024 GRU rows (full batch, t-major)
G3 = 3 * H          # 1536
P = 128

# vocab n-tiles: 62 x 512 + 1 x 256
VT_SIZES = [512] * 62 + [256]
VT_OFFS = np.cumsum([0] + VT_SIZES[:-1]).tolist()


def f32r(ap):
    return ap.bitcast(F32R)


def build(tc, io):
    nc = tc.nc

    const = tc.alloc_tile_pool(name="const", bufs=1)

    # ---------------- persistent constants ----------------
    ident = const.tile([P, P], F32)
    make_identity(nc, ident[:])

    whh = const.tile([P, 4, G3], BF16)
    nc.sync.dma_start(whh[:], io["w_hhT_bf"].rearrange("(k p) g -> p k g", p=P))
    bhhn = const.tile([1, H], BF16)
    nc.sync.dma_start(bhhn[:], io["bhhn_bf"])
    ones_bf = const.tile([1, P], BF16)
    nc.gpsimd.memset(ones_bf[:], 1.0)

    fc2T = const.tile([P, 4, H], F32R)
    nc.sync.dma_start(fc2T[:], io["fc2_wT"].rearrange("(k p) h -> p k h", p=P))
    fcT = const.tile([P, 8, H], F32R)
    nc.sync.dma_start(fcT[:], io["fc_wT"].rearrange("(k p) h -> p k h", p=P))
    fhT = const.tile([P, 4, H], F32R)
    nc.sync.dma_start(fhT[:], io["fh_wT"].rearrange("(k p) h -> p k h", p=P))

    outS = const.tile([P, 2 * H * BL], F32R)   # [s, (bl,d)]
    nc.sync.dma_start(outS[:], io["outS"])

    btot = const.tile([P, 12], F32)
    nc.sync.dma_start(btot[:], io["btot"])
    beT = const.tile([P, 4], F32)
    nc.sync.dma_start(beT[:], io["beT"])
    rbT = const.tile([P, 4], F32)
    nc.sync.dma_start(rbT[:], io["rbT"])
    vP = const.tile([P, 4], F32R)
    nc.sync.dma_start(vP[:], io["vP"])
    maskP = const.tile([P, S], F32)
    nc.sync.dma_start(maskP[:], io["maskP"])

    state0 = const.tile([P, 4, B], F32)
    nc.sync.dma_start(state0[:], io["state0T"].rearrange("p (k b) -> p k b", k=4))

    idx = const.tile([P, 8], I32)
    nc.sync.dma_start(idx[:], io["idx"])

    # persistent big buffers
    gxT = const.tile([P, 12, RB], F32)     # input-gate preacts, transposed
    ssT = const.tile([P, 4, RB], F32R)      # GRU hidden states, transposed
    whT = const.tile([P, 4, S * BL], F32)  # fc1(outputs), transposed
    wssT = const.tile([P, 4, P], F32)      # fc2(ss_local)+be, transposed
    contentT = const.tile([P, 8, P], F32R)
    rT_bf = const.tile([P, 4, P], BF16)
    scoresP = const.tile([P, S], F32)
    attnP = const.tile([P, S], F32)
    attnT = const.tile([P, BL, T], F32R)

    scores_dram = nc.dram_tensor("scores_scratch", [P, S], F32, kind="Internal").ap()

    # ---------------- phase 1: gather x, build xT, gx, wh ----------------
    with tc.tile_pool(name="setup_big", bufs=1) as sbig, \
         tc.tile_pool(name="setup_sb", bufs=3) as spool, \
         tc.tile_pool(name="setup_ps", bufs=2, space="PSUM") as spsum, \
         tc.tile_pool(name="wih_sb", bufs=4) as wpool:
        xT = sbig.tile([P, 4, RB], F32R, tag="xT")
        fc1T = sbig.tile([P, 8, H], F32R, tag="fc1T")
        nc.sync.dma_start(fc1T[:], io["fc1_wT"].rearrange("(k p) h -> p k h", p=P))
        outTt = sbig.tile([P, 8, S * BL], F32R, tag="outTt")   # [2H-chunk, (s,bl)]
        nc.sync.dma_start(outTt[:], io["outT"].rearrange("(k p) c -> p k c", p=P))

        for i in range(8):
            xg = spool.tile([P, E], F32, tag="xg")
            nc.gpsimd.indirect_dma_start(
                out=xg[:],
                out_offset=None,
                in_=io["emb"],
                in_offset=IndirectOffsetOnAxis(ap=idx[:, i : i + 1], axis=0),
            )
            ps = spsum.tile([P, E], F32, tag="xtps")
            for k in range(4):
                nc.tensor.transpose(
                    out=ps[:, ts(k, P)], in_=xg[:, ts(k, P)], identity=ident[:]
                )
            nc.vector.tensor_copy(
                out=xT[:, :, ts(i, P)],
                in_=ps[:].rearrange("p (k c) -> p k c", k=4),
            )

        # gxT = w_ih @ x.T + btot  (n-gate chunks carry b_ih only; r,z get b_ih+b_hh)
        for m in range(12):
            ps0 = spsum.tile([P, 512], F32, tag="gx0")
            ps1 = spsum.tile([P, 512], F32, tag="gx1")
            for k in range(4):
                wi = wpool.tile([P, P], F32R, tag="wi")
                nc.sync.dma_start(wi[:], io["w_ihT"][ts(k, P), ts(m, P)])
                nc.tensor.matmul(
                    out=ps0[:], lhsT=f32r(wi[:]), rhs=f32r(xT[:, k, 0:512]),
                    start=(k == 0), stop=(k == 3),
                )
                nc.tensor.matmul(
                    out=ps1[:], lhsT=f32r(wi[:]), rhs=f32r(xT[:, k, 512:1024]),
                    start=(k == 0), stop=(k == 3),
                )
            nc.scalar.activation(
                out=gxT[:, m, 0:512], in_=ps0[:], func=AF.Identity,
                bias=btot[:, m : m + 1],
            )
            nc.scalar.activation(
                out=gxT[:, m, 512:1024], in_=ps1[:], func=AF.Identity,
                bias=btot[:, m : m + 1],
            )

        # whT = fc1_w @ outputs_loc.T   (bias folded into wssT later)
        for m in range(4):
            ps = spsum.tile([P, S * BL], F32, tag="whps")
            for k in range(8):
                nc.tensor.matmul(
                    out=ps[:], lhsT=f32r(fc1T[:, k, ts(m, P)]), rhs=f32r(outTt[:, k, :]),
                    start=(k == 0), stop=(k == 7),
                )
            nc.scalar.copy(out=whT[:, m, :], in_=ps[:])

    # ---------------- phase 2: GRU scan (full B, transposed layout) ----------------
    with tc.tile_pool(name="scan_sb", bufs=3) as kpool, \
         tc.tile_pool(name="scan_ps", bufs=2, space="PSUM") as kpsum:
        hbf = kpool.tile([P, 4, B], BF16, tag="hbf")
        nc.vector.tensor_copy(out=hbf[:], in_=state0[:])
        for t in range(T):
            hprev = state0[:] if t == 0 else ssT[:, :, ts(t - 1, B)]
            ghT = kpsum.tile([P, 12 * B], F32, tag="ghT")
            # gate chunk order: r (m 0-3), z (4-7), n (8-11)
            for m in range(12):
                for k in range(4):
                    nc.tensor.matmul(
                        out=ghT[:, ts(m, B)],
                        lhsT=whh[:, k, ts(m, P)],
                        rhs=hbf[:, k, :],
                        start=(k == 0),
                        stop=(k == 3 and m < 8),
                    )
                if m >= 8:
                    # + b_hh for the n gate (rank-1 ones row), pre r-multiply
                    nc.tensor.matmul(
                        out=ghT[:, ts(m, B)],
                        lhsT=bhhn[:, ts(m - 8, P)],
                        rhs=ones_bf[:, 0:B],
                        start=False,
                        stop=True,
                    )
            if t == 0 and "dbg_gh0" in io:
                gh0 = kpool.tile([P, 12 * B], F32, tag="gh0dbg")
                nc.vector.tensor_copy(out=gh0[:], in_=ghT[:])
                nc.sync.dma_start(io["dbg_gh0"], gh0[:])
            rzp = kpool.tile([P, 8, B], F32, tag="rzp")
            nc.vector.tensor_add(
                out=rzp[:],
                in0=ghT[:, 0 : 8 * B].rearrange("p (m b) -> p m b", b=B),
                in1=gxT[:, 0:8, ts(t, B)],
            )
            rz = kpool.tile([P, 8, B], F32, tag="rz")
            nc.scalar.activation(out=rz[:], in_=rzp[:], func=AF.Sigmoid)
            nt1 = kpool.tile([P, 4, B], F32, tag="nt1")
            nc.vector.tensor_mul(
                out=nt1[:], in0=rz[:, 0:4, :],
                in1=ghT[:, 8 * B : 12 * B].rearrange("p (m b) -> p m b", b=B),
            )
            nt2 = kpool.tile([P, 4, B], F32, tag="nt2")
            nc.vector.tensor_add(out=nt2[:], in0=nt1[:], in1=gxT[:, 8:12, ts(t, B)])
            nT = kpool.tile([P, 4, B], F32, tag="nT")
            nc.scalar.activation(out=nT[:], in_=nt2[:], func=AF.Tanh)
            dT = kpool.tile([P, 4, B], F32, tag="dT")
            nc.vector.tensor_sub(out=dT[:], in0=hprev, in1=nT[:])
            eT = kpool.tile([P, 4, B], F32, tag="eT")
            nc.vector.tensor_mul(out=eT[:], in0=rz[:, 4:8, :], in1=dT[:])
            nc.vector.tensor_add(out=ssT[:, :, ts(t, B)], in0=nT[:], in1=eT[:])
            hbf = kpool.tile([P, 4, B], BF16, tag="hbf")
            nc.vector.tensor_copy(out=hbf[:], in_=ssT[:, :, ts(t, B)])

    # final hidden state out
    nc.sync.dma_start(io["hT_out"], ssT[:, :, ts(T - 1, B)])

    if "dbg_ssT" in io:
        nc.sync.dma_start(io["dbg_ssT"], ssT[:])
        nc.sync.dma_start(io["dbg_gxT"], gxT[:])

    # ---------------- phase 3: attention ----------------
    with tc.tile_pool(name="attn_sb", bufs=2) as apool, \
         tc.tile_pool(name="attn_ps", bufs=1, space="PSUM") as apsum, \
         tc.tile_pool(name="dot_ps", bufs=3, space="PSUM") as dpsum:
        # wssT = fc2_w @ ss_local.T + (fc1_b + fc2_b)
        for m in range(4):
            ps = apsum.tile([P, P], F32, tag="wssps")
            for k in range(4):
                rhs = ssT[:, k, :].rearrange("p (t b) -> p b t", b=B)[:, 0:BL, :]
                nc.tensor.matmul(
                    out=ps[:], lhsT=f32r(fc2T[:, k, ts(m, P)]), rhs=f32r(rhs),
                    start=(k == 0), stop=(k == 3),
                )
            nc.scalar.activation(
                out=wssT[:, m, :], in_=ps[:], func=AF.Identity, bias=beT[:, m : m + 1]
            )

        # e = tanh(whT + wssT); scores = v . e  (per (bl, t-quad))
        for bl in range(BL):
            whTb = [
                whT[:, m, :].rearrange("p (s b) -> p b s", b=BL)[:, bl, :]
                for m in range(4)
            ]
            for tq in range(16):
                et = apool.tile([P, 4, 4, S], F32R, tag="etan")
                for m in range(4):
                    for tsub in range(4):
                        pcol = bl * T + tq * 4 + tsub
                        nc.vector.tensor_scalar(
                            out=et[:, m, tsub, :], in0=whTb[m],
                            scalar1=wssT[:, m, pcol : pcol + 1], scalar2=None,
                            op0=ALU.add,
                        )
                nc.scalar.activation(out=et[:], in_=et[:], func=AF.Tanh)
                dps = dpsum.tile([1, 4 * S], F32, tag="dotps")
                for tsub in range(4):
                    for k in range(4):
                        nc.tensor.matmul(
                            out=dps[:, ts(tsub, S)],
                            lhsT=f32r(vP[:, k : k + 1]),
                            rhs=f32r(et[:, k, tsub, :]),
                            start=(k == 0), stop=(k == 3),
                        )
                stg = apool.tile([1, 4 * S], F32, tag="dstage")
                if tq % 2 == 0:
                    nc.scalar.copy(out=stg[:], in_=dps[:])
                else:
                    nc.vector.tensor_copy(out=stg[:], in_=dps[:])
                r0 = bl * T + tq * 4
                nc.sync.dma_start(
                    scores_dram[r0 : r0 + 4, :].rearrange("t s -> (t s)").unsqueeze(0),
                    stg[:],
                )

        # softmax over s with additive mask
        nc.sync.dma_start(scoresP[:], scores_dram)
        sc2 = apool.tile([P, S], F32, tag="sc2")
        nc.vector.tensor_add(out=sc2[:], in0=scoresP[:], in1=maskP[:])
        mxn = apool.tile([P, 1], F32, tag="mxn")
        nc.vector.tensor_reduce(
            out=mxn[:], in_=sc2[:], axis=AX.X, op=ALU.max, negate=True
        )
        ex = apool.tile([P, S], F32, tag="ex")
        sm = apool.tile([P, 1], F32, tag="sm")
        nc.scalar.activation(
            out=ex[:], in_=sc2[:], func=AF.Exp, bias=mxn[:, 0:1], accum_out=sm[:]
        )
        rs = apool.tile([P, 1], F32, tag="rs")
        nc.vector.reciprocal(out=rs[:], in_=sm[:])
        nc.vector.tensor_scalar(
            out=attnP[:], in0=ex[:], scalar1=rs[:, 0:1], scalar2=None, op0=ALU.mult
        )

        # attn.T per local batch (PE transpose; bl=1 staged to partition base 0)
        atmp = apool.tile([T, S], F32, tag="atmp")
        for bl in range(BL):
            if bl == 0:
                src = attnP[0:T, :]
            else:
                nc.sync.dma_start(atmp[:], attnP[T : 2 * T, :])
                src = atmp[:]
            tps = apsum.tile([P, T], F32, tag="atps")
            nc.tensor.transpose(out=tps[:], in_=src, identity=ident[0:T, 0:T])
            nc.vector.tensor_copy(out=attnT[:, bl, :], in_=tps[:])

        # contentT = outputs_loc.T @ attn.T
        for mc in range(8):
            ps = apsum.tile([P, BL * T], F32, tag="ctps")
            for bl in range(BL):
                nc.tensor.matmul(
                    out=ps[:, ts(bl, T)],
                    lhsT=f32r(outS[:, bl * 2 * H + mc * P : bl * 2 * H + (mc + 1) * P]),
                    rhs=f32r(attnT[:, bl, :]),
                    start=True, stop=True,
                )
            nc.scalar.copy(out=contentT[:, mc, :], in_=ps[:])

        # rT = tanh(fc_w @ content.T + fh_w @ ss_local.T + rb)
        for m in range(4):
            ps = apsum.tile([P, P], F32, tag="rps")
            for k in range(8):
                nc.tensor.matmul(
                    out=ps[:], lhsT=f32r(fcT[:, k, ts(m, P)]), rhs=f32r(contentT[:, k, :]),
                    start=(k == 0), stop=False,
                )
            for k in range(4):
                rhs = ssT[:, k, :].rearrange("p (t b) -> p b t", b=B)[:, 0:BL, :]
                nc.tensor.matmul(
                    out=ps[:], lhsT=f32r(fhT[:, k, ts(m, P)]), rhs=f32r(rhs),
                    start=False, stop=(k == 3),
                )
            nc.scalar.activation(
                out=rT_bf[:, m, :], in_=ps[:], func=AF.Tanh, bias=rbT[:, m : m + 1]
            )

    if "dbg_ssT" in io:
        nc.sync.dma_start(io["dbg_whT"], whT[:])
        nc.sync.dma_start(io["dbg_wssT"], wssT[:])
        nc.sync.dma_start(io["dbg_scores"], scoresP[:])
        nc.sync.dma_start(io["dbg_attn"], attnP[:])
        nc.sync.dma_start(io["dbg_contentT"], contentT[:])

    # ---------------- phase 4: vocab projection ----------------
    with tc.tile_pool(name="voc_sb", bufs=6) as vpool, \
         tc.tile_pool(name="vocb_sb", bufs=4) as vbpool, \
         tc.tile_pool(name="voc_ps", bufs=6, space="PSUM") as vpsum:
        wsT = io["ws_wT_bf"].rearrange("(k p) v -> p k v", p=P)
        for i, (off, nsz) in enumerate(zip(VT_OFFS, VT_SIZES)):
            wst = vpool.tile([P, 4, nsz], BF16, tag="wst")
            nc.sync.dma_start(wst[:], wsT[:, :, off : off + nsz])
            wsbt = vbpool.tile([1, nsz], BF16, tag="wsbt")
            nc.sync.dma_start(wsbt[:], io["ws_b_bf"][0:1, off : off + nsz])
            ps = vpsum.tile([P, nsz], F32, tag="vps")
            nc.tensor.matmul(
                out=ps[:], lhsT=ones_bf[:], rhs=wsbt[:], start=True, stop=False
            )
            for k in range(4):
                nc.tensor.matmul(
                    out=ps[:], lhsT=rT_bf[:, k, :], rhs=wst[:, k, :],
                    start=False, stop=(k == 3),
                )
            vout = vbpool.tile([P, nsz], F32, tag="vout")
            if i % 2 == 0:
                nc.scalar.copy(out=vout[:], in_=ps[:])
            else:
                nc.vector.tensor_copy(out=vout[:], in_=ps[:])
            nc.sync.dma_start(io["logits"][:, off : off + nsz], vout[:])

    const.release()


def _dram_inputs(nc):
    """Declare all per-core DRAM inputs; returns dict name -> AP."""
    d = {}

    def inp(name, shape, dt):
        d[name] = nc.dram_tensor(name, list(shape), dt, kind="ExternalInput").ap()

    inp("idx", [P, 8], I32)
    inp("emb", [V, E], F32)
    inp("state0T", [P, 4 * B], F32)
    inp("w_ihT", [E, G3], F32R)
    inp("w_hhT_bf", [H, G3], BF16)
    inp("btot", [P, 12], F32)
    inp("bhhn_bf", [1, H], BF16)
    inp("outT", [2 * H, S * BL], F32R)
    inp("outS", [P, 2 * H * BL], F32R)
    inp("fc1_wT", [2 * H, H], F32R)
    inp("fc2_wT", [H, H], F32R)
    inp("beT", [P, 4], F32)
    inp("vP", [P, 4], F32R)
    inp("maskP", [P, S], F32)
    inp("fc_wT", [2 * H, H], F32R)
    inp("fh_wT", [H, H], F32R)
    inp("rbT", [P, 4], F32)
    inp("ws_wT_bf", [H, V], BF16)
    inp("ws_b_bf", [1, V], BF16)

    d["logits"] = nc.dram_tensor("logits", [P, V], F32, kind="ExternalOutput").ap()
    d["hT_out"] = nc.dram_tensor("hT_out", [P, 4, B], F32R, kind="ExternalOutput").ap()
    if os.environ.get("KERNEL_DEBUG", "0") == "1":
        for nm, shp, dt_ in [("dbg_gh0", [P, 12 * B], F32),
                        ("dbg_ssT", [P, 4, RB], F32R), ("dbg_gxT", [P, 12, RB], F32),
                        ("dbg_whT", [P, 4, S * BL], F32), ("dbg_wssT", [P, 4, P], F32),
                        ("dbg_scores", [P, S], F32), ("dbg_attn", [P, S], F32),
                        ("dbg_contentT", [P, 8, P], F32R)]:
            d[nm] = nc.dram_tensor(nm, shp, dt_, kind="ExternalOutput").ap()
    return d


_COMPILED = {}


def _get_program():
    if "nc" not in _COMPILED:
        nc = bacc.Bacc("TRN2", target_bir_lowering=False, debug=False,
                       num_devices=NCORES)
        with tile.TileContext(nc) as tc:
            io = _dram_inputs(nc)
            build(tc, io)
        nc.compile()
        _COMPILED["nc"] = nc
    return _COMPILED["nc"]


def _host_prep(tgt, state, outputs, src_len, emb, w_ih, w_hh, b_ih, b_hh,
               fc1_w, fc1_b, fc2_w, fc2_b, v_w, fc_w, fc_b, fh_w, fh_b,
               ws_w, ws_b):
    """Build per-core input dicts (all numpy, C-contiguous)."""
    f32 = np.float32
    bf16 = ml_dtypes.bfloat16

    tgt = np.asarray(tgt).astype(np.int32)
    state = np.asarray(state, dtype=f32)
    outputs = np.asarray(outputs, dtype=f32)
    src_len = np.asarray(src_len).astype(np.int64)
    emb = np.ascontiguousarray(np.asarray(emb, dtype=f32))
    w_ih = np.asarray(w_ih, dtype=f32); w_hh = np.asarray(w_hh, dtype=f32)
    b_ih = np.asarray(b_ih, dtype=f32); b_hh = np.asarray(b_hh, dtype=f32)
    fc1_w = np.asarray(fc1_w, dtype=f32); fc1_b = np.asarray(fc1_b, dtype=f32)
    fc2_w = np.asarray(fc2_w, dtype=f32); fc2_b = np.asarray(fc2_b, dtype=f32)
    v_w = np.asarray(v_w, dtype=f32)
    fc_w = np.asarray(fc_w, dtype=f32); fc_b = np.asarray(fc_b, dtype=f32)
    fh_w = np.asarray(fh_w, dtype=f32); fh_b = np.asarray(fh_b, dtype=f32)
    ws_w = np.asarray(ws_w, dtype=f32); ws_b = np.asarray(ws_b, dtype=f32)

    # shared (batch-independent) arrays
    w_ihT = np.ascontiguousarray(w_ih.T)                       # [E, 3H]
    w_hhT_bf = np.ascontiguousarray(w_hh.T).astype(bf16)       # [H, 3H]
    btot_full = (b_ih + b_hh).copy()
    btot_full[2 * H :] = b_ih[2 * H :]                         # n chunks: b_ih only
    btot = np.ascontiguousarray(btot_full.reshape(12, P).T)    # [128, 12]
    bhhn_bf = np.ascontiguousarray(b_hh[2 * H :].reshape(1, H)).astype(bf16)
    fc1_wT = np.ascontiguousarray(fc1_w.T)                     # [2H, H]
    fc2_wT = np.ascontiguousarray(fc2_w.T)                     # [H, H]
    beT = np.ascontiguousarray((fc1_b + fc2_b).reshape(4, P).T)
    vP = np.ascontiguousarray(v_w[0].reshape(4, P).T)
    fc_wT = np.ascontiguousarray(fc_w.T)
    fh_wT = np.ascontiguousarray(fh_w.T)
    rbT = np.ascontiguousarray((fc_b + fh_b).reshape(4, P).T)
    ws_wT_bf = np.ascontiguousarray(ws_w.T).astype(bf16)       # [H, V]
    ws_b_bf = ws_b.reshape(1, V).astype(bf16)

    in_maps = []
    for c in range(NCORES):
        b0 = 2 * c
        rest = [b for b in range(B) if b not in (b0, b0 + 1)]
        perm = np.array([b0, b0 + 1] + rest)

        tgt_p = tgt[:, perm]                                   # (T, B) t-major rows
        idx_arr = np.ascontiguousarray(tgt_p.reshape(RB).reshape(8, P).T).astype(
            np.int32
        )

        st = state[0][perm]                                    # (B, H)
        state0T = np.ascontiguousarray(
            st.T.reshape(4, P, B).transpose(1, 0, 2).reshape(P, 4 * B)
        )

        out_loc = outputs[:, [b0, b0 + 1], :]                  # (S, 2, 2H)
        outT = np.ascontiguousarray(
            out_loc.transpose(2, 0, 1).reshape(2 * H, S * BL)
        )                                                      # col = s*2+bl
        outS = np.ascontiguousarray(out_loc.reshape(P, 2 * H * BL))

        maskP = np.zeros((P, S), dtype=f32)
        for bl in range(BL):
            sl = int(src_len[b0 + bl])
            maskP[bl * T : (bl + 1) * T, sl:] = -1e9

        in_maps.append(dict(
            idx=idx_arr, emb=emb, state0T=state0T, w_ihT=w_ihT,
            w_hhT_bf=w_hhT_bf, btot=btot, bhhn_bf=bhhn_bf, outT=outT,
            outS=outS, fc1_wT=fc1_wT, fc2_wT=fc2_wT, beT=beT, vP=vP,
            maskP=maskP, fc_wT=fc_wT, fh_wT=fh_wT, rbT=rbT,
            ws_wT_bf=ws_wT_bf, ws_b_bf=ws_b_bf,
        ))
    return in_maps


LAST_EXEC_NS = None
LAST_RESULTS = None


def kernel(tgt, state, outputs, src_len, teacher_forcing, emb, w_ih, w_hh,
           b_ih, b_hh, fc1_w, fc1_b, fc2_w, fc2_b, v_w, v_b, fc_w, fc_b,
           fh_w, fh_b, ws_w, ws_b):
    global LAST_EXEC_NS, LAST_RESULTS
    del teacher_forcing, v_b  # teacher-forcing branch fixed; v_b cancels in softmax

    nc = _get_program()
    in_maps = _host_prep(tgt, state, outputs, src_len, emb, w_ih, w_hh, b_ih,
                         b_hh, fc1_w, fc1_b, fc2_w, fc2_b, v_w, fc_w, fc_b,
                         fh_w, fh_b, ws_w, ws_b)
    trace = os.environ.get("KERNEL_TRACE", "0") == "1"
    res = run_bass_kernel_spmd(
        nc, in_maps, core_ids=list(range(NCORES)), trace=trace,
    )
    LAST_EXEC_NS = res.exec_time_ns
    LAST_RESULTS = res

    logits = np.empty((T, B, V), dtype=np.float32)
    for c in range(NCORES):
        loc = res.results[c]["logits"].reshape(BL, T, V)
        logits[:, 2 * c : 2 * c + 2, :] = loc.transpose(1, 0, 2)
    hT = res.results[0]["hT_out"].reshape(P, 4, B).transpose(1, 0, 2).reshape(H, B)
    hT = np.ascontiguousarray(hT.T)[None]                      # (1, B, H)
    return logits, hT
